# revision 1
# baseline (speedup 1.0000x reference)
"""Multi-head attention kernel for Trainium2, 8 NeuronCores, data-parallel over batch.

Problem: batch=16, pos=577, d_model=1024, n_heads=16, d_head=64, fp32.
Sharding: batch across 8 cores (2 batch items per core), no collectives.

v2: bf16 compute throughout (fp32 PSUM accumulation), host-marshaled
transposed/bf16 input layouts for contiguous DMA, biases fused into
PSUM-eviction DVE ops (no bias matmuls).

Per-core pipeline (B=2 local batch, T=577, D=1024, H=16, E=64):
  A) QKV projections from X^T [D, B*T] bf16 and W^T [D, HE] bf16.
     Q^T,K^T as [HE, B*T] bf16 (+b_Q/b_K per-partition scalar-add on evict);
     V as [B*T, HE] augmented per head [V_h | 1] bf16 (b_V deferred to Z:
     softmax weights sum to 1, so P(V+b) = PV + b).
  B) Per (batch, head-pair): S^T = K_h @ Q_h^T row-paired K=64 matmuls,
     P\' = exp(S^T/8) on ACT -> bf16 (no max subtraction; scores O(1)).
     Z\'aug^T = [V_h|1]^T @ P\'^T -> [65, T]; row 64 = softmax denominator.
     Evict Z\' with +b_V scalar-add; reciprocal of denom; broadcast via
     DRAM-roundtrip DMA; bf16 multiply.
  C) Out = Z^T.T @ W_O (K=128 accumulation over 8 head-pairs) + b_O via
     DVE tensor_add against a DMA-broadcast b_O tile.
"""
import numpy as np

import concourse.bass as bass
import concourse.tile as tile
from concourse import bacc, mybir

F32 = mybir.dt.float32
BF16 = mybir.dt.bfloat16
AF = mybir.ActivationFunctionType

NCORES = 8
B = 2            # batch per core
T = 577
D = 1024
H = 16
E = 64
HE = H * E       # 1024
BT = B * T       # 1154

KT = 8                                   # k-tiles over D
MT = 8                                   # m-tiles over HE (head pairs)
A_N = [(0, 386), (386, 384), (770, 384)]  # bt chunks for phase A
TT = [(0, 128), (128, 128), (256, 128), (384, 128), (512, 65)]  # tiles over T
QN = [(0, 289), (289, 288)]              # q chunks (1-bank PSUM tiles)
N512 = [(0, 512), (512, 512)]            # 512-chunks over HE / D
VW = E + 1                               # 65: augmented V width per head


def build_graph():
    nc = bacc.Bacc("TRN2", target_bir_lowering=False, debug=False,
                   num_devices=NCORES)

    xq = nc.dram_tensor("query_input", [D, BT], BF16, kind="ExternalInput")
    xk = nc.dram_tensor("key_input", [D, BT], BF16, kind="ExternalInput")
    xv = nc.dram_tensor("value_input", [D, BT], BF16, kind="ExternalInput")
    wq = nc.dram_tensor("W_Q", [D, HE], BF16, kind="ExternalInput")
    wk = nc.dram_tensor("W_K", [D, HE], BF16, kind="ExternalInput")
    wv = nc.dram_tensor("W_V", [D, HE], BF16, kind="ExternalInput")
    wo = nc.dram_tensor("W_O", [HE, D], BF16, kind="ExternalInput")
    bq = nc.dram_tensor("b_Q", [128, MT], F32, kind="ExternalInput")
    bk = nc.dram_tensor("b_K", [128, MT], F32, kind="ExternalInput")
    bv = nc.dram_tensor("b_V", [128, MT], F32, kind="ExternalInput")
    bo = nc.dram_tensor("b_O", [1, D], F32, kind="ExternalInput")
    out = nc.dram_tensor("out", [B, T, D], F32, kind="ExternalOutput")

    with tile.TileContext(nc) as tc:
        _body(nc, tc, xq, xk, xv, wq, wk, wv, wo, bq, bk, bv, bo, out)
    nc.compile()
    return nc


def _body(nc, tc, xq, xk, xv, wq, wk, wv, wo, bq, bk, bv, bo, out):
    from contextlib import ExitStack
    est = ExitStack()
    with est:
        # ---- persistent pools; packed tiles ----
        sbQ_p = est.enter_context(tc.tile_pool(name="sbQ", bufs=1))
        sbK_p = est.enter_context(tc.tile_pool(name="sbK", bufs=1))
        sbVg_p = est.enter_context(tc.tile_pool(name="sbVg", bufs=1))
        sbZ_p = est.enter_context(tc.tile_pool(name="sbZ", bufs=1))
        xt_p = est.enter_context(tc.tile_pool(name="xt", bufs=3))
        wt_p = est.enter_context(tc.tile_pool(name="wt", bufs=3))
        const_p = est.enter_context(tc.tile_pool(name="const", bufs=1))
        dram_p = est.enter_context(tc.tile_pool(name="dramd", bufs=1, space="DRAM"))

        bqc = const_p.tile([128, MT], F32, tag="bqc")
        bkc = const_p.tile([128, MT], F32, tag="bkc")
        bvc = const_p.tile([128, MT], F32, tag="bvc")
        boc = const_p.tile([128, D], F32, tag="boc")
        nc.sync.dma_start(bqc[:], bq.ap())
        nc.sync.dma_start(bkc[:], bk.ap())
        nc.sync.dma_start(bvc[:], bv.ap())
        nc.sync.dma_start(boc[:], bo.ap().partition_broadcast(128))

        # packed persistent tiles (bf16)
        sbQ = sbQ_p.tile([128, MT * BT], BF16, tag="sbQ")     # [:, m*BT + bt]
        sbK = sbK_p.tile([128, MT * BT], BF16, tag="sbK")
        sbVg = sbVg_p.tile([128, 10 * H * VW], BF16, tag="sbVg")
        sbZ = sbZ_p.tile([128, B * MT * T], BF16, tag="sbZ")  # [:, (b*MT+hp)*T + t]

        def zsl(b, hp, lo, sz, to, tsz):
            base = (b * MT + hp) * T
            return sbZ[lo:lo + sz, base + to:base + to + tsz]

        # ================= Phase A: projections =================
        def load_xw(x_in, w_in):
            xt = xt_p.tile([128, KT * BT], BF16, tag="xt", name="xt")
            wt = wt_p.tile([128, KT * HE], BF16, tag="wt", name="wt")
            for k in range(KT):
                nc.sync.dma_start(xt[:, k * BT:(k + 1) * BT],
                                  x_in.ap()[k * 128:(k + 1) * 128, :])
                nc.sync.dma_start(wt[:, k * HE:(k + 1) * HE],
                                  w_in.ap()[k * 128:(k + 1) * 128, :])
            return xt, wt

        # --- V first: frees its PSUM scope before the A/B overlap region ---
        with tc.tile_pool(name="psV", bufs=6, space="PSUM") as psV_p:
            xt, wt = load_xw(xv, wv)
            for b in range(B):
                for ti, (to, tsz) in enumerate(TT):
                    vt = b * 5 + ti
                    vbase = vt * H * VW
                    bto = b * T + to
                    for ni, (no, nsz) in enumerate(N512):
                        ps = psV_p.tile([128, 512], F32, tag="psV", name="psV")
                        for k in range(KT):
                            nc.tensor.matmul(
                                ps[:tsz, :],
                                xt[:, k * BT + bto:k * BT + bto + tsz],
                                wt[:, k * HE + no:k * HE + no + nsz],
                                start=(k == 0), stop=(k == KT - 1))
                        for hh in range(8):
                            h = ni * 8 + hh
                            nc.vector.tensor_copy(
                                sbVg[:tsz, vbase + h * VW:vbase + h * VW + E],
                                ps[:tsz, hh * E:hh * E + E])
                    onecols = sbVg[:tsz, vbase:vbase + H * VW].rearrange(
                        "p (h c) -> p h c", c=VW)[:, :, E:E + 1]
                    nc.vector.memset(onecols, 1.0)

        # --- Q/K interleaved per m-tile; overlaps with phase B below ---
        psA_p = None  # allocated in the A/B scope below
        xtq, wtq = load_xw(xq, wq)
        xtk, wtk = load_xw(xk, wk)
        # ========== Phase B body (emitted interleaved with Q/K below) ==========
        from contextlib import ExitStack as _ES
        ab_scope = _ES()
        psA_p = ab_scope.enter_context(tc.tile_pool(name="psA", bufs=2, space="PSUM"))
        sbP_p = ab_scope.enter_context(tc.tile_pool(name="sbP", bufs=3))
        rp_p = ab_scope.enter_context(tc.tile_pool(name="rp", bufs=2))
        psS_p = ab_scope.enter_context(tc.tile_pool(name="psS", bufs=2, space="PSUM"))
        psZ_p = ab_scope.enter_context(tc.tile_pool(name="psZ", bufs=1, space="PSUM"))

        def emit_attn(b, hp):
            bt0 = b * T
            qb = hp * BT + bt0
            # rp[hd]: row 0 collects denominators across both q chunks
            rp = [rp_p.tile([128, T], F32, tag=f"rp{hd}", name=f"rp{hd}")
                  for hd in range(2)]
            for qi, (qo, qsz) in enumerate(QN):
                psz = [psZ_p.tile([65, 289], F32, tag=f"psz{hd}",
                                  name=f"psz{hd}")
                       for hd in range(2)]
                for kt, (ko, ksz) in enumerate(TT):
                    vbase = (b * 5 + kt) * H * VW
                    pp = sbP_p.tile([128, 2 * 289], BF16,
                                    tag=f"pk{kt % 2}", name=f"pp{kt % 2}")
                    for hd in range(2):
                        lo = hd * 64
                        pss = psS_p.tile([128, 289], F32, tag=f"psS{hd}",
                                         name=f"pss{hd}")
                        nc.tensor.matmul(
                            pss[:ksz, :qsz],
                            sbK[lo:lo + 64, qb + ko:qb + ko + ksz],
                            sbQ[lo:lo + 64, qb + qo:qb + qo + qsz],
                            start=True, stop=True, tile_position=(lo, 0))
                        nc.scalar.activation(
                            pp[:ksz, hd * 289:hd * 289 + qsz],
                            pss[:ksz, :qsz], AF.Exp, scale=0.125)
                    for hd in range(2):
                        h = 2 * hp + hd
                        nc.tensor.matmul(
                            psz[hd][:, :qsz],
                            sbVg[:ksz, vbase + h * VW:vbase + h * VW + VW],
                            pp[:ksz, hd * 289:hd * 289 + qsz],
                            start=(kt == 0), stop=(kt == 4))
                # ---- evict Z' (+b_V) and denominator row ----
                for hd in range(2):
                    lo = hd * 64
                    nc.vector.tensor_scalar_add(
                        zsl(b, hp, lo, 64, qo, qsz), psz[hd][0:64, :qsz],
                        bvc[lo:lo + 64, hp:hp + 1])
                    nc.vector.tensor_copy(
                        rp[hd][0:1, qo:qo + qsz], psz[hd][64:65, :qsz])
            # ---- normalize (after both q chunks) ----
            for hd in range(2):
                h = 2 * hp + hd
                lo = hd * 64
                nc.vector.reciprocal_approx_fast(rp[hd][0:1, :], rp[hd][0:1, :])
                rpb = rp_p.tile([128, T], BF16, tag=f"rpb{hd}", name=f"rpb{hd}")
                nc.vector.tensor_copy(rpb[0:1, :], rp[hd][0:1, :])
                dd = dram_p.tile([1, T], BF16, tag=f"d{b * H + h}",
                                 name=f"dd{b * H + h}")
                nc.sync.dma_start(dd[:], rpb[0:1, :])
                nc.sync.dma_start(rpb[lo:lo + 64, :],
                                  dd[:].partition_broadcast(64))
                nc.vector.tensor_mul(zsl(b, hp, lo, 64, 0, T),
                                     zsl(b, hp, lo, 64, 0, T),
                                     rpb[lo:lo + 64, :])

        for m in range(MT):
            for (xt, wt, b_col, dest) in ((xtq, wtq, bqc, sbQ),
                                          (xtk, wtk, bkc, sbK)):
                for (no, nsz) in A_N:
                    ps = psA_p.tile([128, 386], F32, tag="psA", name="psA")
                    for k in range(KT):
                        nc.tensor.matmul(
                            ps[:, :nsz],
                            wt[:, k * HE + m * 128:k * HE + (m + 1) * 128],
                            xt[:, k * BT + no:k * BT + no + nsz],
                            start=(k == 0), stop=(k == KT - 1))
                    nc.vector.tensor_scalar_add(
                        dest[:, m * BT + no:m * BT + no + nsz],
                        ps[:, :nsz], b_col[:, m:m + 1])
            if m < MT - 1:
                for b in range(B):
                    emit_attn(b, m)
        # final head-pairs interleaved with phase C per batch (emitted below)

        # ================= Phase C: output projection =================
        wot = wt_p.tile([128, MT * D], BF16, tag="wt", name="wot")
        for hp in range(MT):
            nc.sync.dma_start(wot[:, hp * D:(hp + 1) * D],
                              wo.ap()[hp * 128:(hp + 1) * 128, :])

        def emit_out(b, psO_p):
            for (mo, msz) in TT:
                for (no, nsz) in N512:
                    ps = psO_p.tile([128, 512], F32, tag="psO", name="psO")
                    for hp in range(MT):
                        nc.tensor.matmul(
                            ps[:msz, :],
                            zsl(b, hp, 0, 128, mo, msz),
                            wot[:, hp * D + no:hp * D + no + nsz],
                            start=(hp == 0), stop=(hp == MT - 1))
                    so = sbO_p.tile([128, 512], F32, tag="sbO", name="sbO")
                    nc.vector.tensor_add(so[:msz, :], ps[:msz, :],
                                         boc[:msz, no:no + nsz])
                    nc.sync.dma_start(
                        out.ap()[b, mo:mo + msz, no:no + nsz], so[:msz, :])

        emit_attn(0, MT - 1)
        emit_attn(1, MT - 1)
        ab_scope.close()
        sbO_p = est.enter_context(tc.tile_pool(name="sbO", bufs=3))
        with tc.tile_pool(name="psO", bufs=4, space="PSUM") as psO_p:
            emit_out(0, psO_p)
            emit_out(1, psO_p)


_GRAPH = None


def _get_graph():
    global _GRAPH
    if _GRAPH is None:
        _GRAPH = build_graph()
    return _GRAPH


def kernel(query_input, key_input, value_input, W_Q, W_K, W_V, W_O,
           b_Q, b_K, b_V, b_O, _trace=False, _trace_kwargs=None):
    import ml_dtypes
    from concourse.bass_utils import run_bass_kernel_spmd

    nc = _get_graph()
    f = np.ascontiguousarray
    bf = ml_dtypes.bfloat16

    def xT(x, sl):
        x = np.asarray(x[sl], np.float32)
        return f(x.reshape(B * T, D).T.astype(bf))

    def wT(w):
        w = np.asarray(w, np.float32)
        return f(w.transpose(1, 0, 2).reshape(D, HE).astype(bf))

    def bcol(bx):
        bx = np.asarray(bx, np.float32).reshape(HE)
        return f(bx.reshape(MT, 128).T)

    wq_m, wk_m, wv_m = wT(W_Q), wT(W_K), wT(W_V)
    wo_m = f(np.asarray(W_O, np.float32).reshape(HE, D).astype(bf))
    bq_m, bk_m, bv_m = bcol(b_Q), bcol(b_K), bcol(b_V)
    bo_m = f(np.asarray(b_O, np.float32).reshape(1, D))
    in_maps = []
    for c in range(NCORES):
        sl = slice(2 * c, 2 * c + 2)
        in_maps.append({
            "query_input": xT(query_input, sl),
            "key_input": xT(key_input, sl),
            "value_input": xT(value_input, sl),
            "W_Q": wq_m,
            "W_K": wk_m,
            "W_V": wv_m,
            "W_O": wo_m,
            "b_Q": bq_m,
            "b_K": bk_m,
            "b_V": bv_m,
            "b_O": bo_m,
        })
    res = run_bass_kernel_spmd(nc, in_maps, core_ids=list(range(NCORES)),
                               trace=_trace, **(_trace_kwargs or {}))
    outp = np.concatenate([res.results[c]["out"] for c in range(NCORES)], axis=0)
    if _trace:
        kernel._last_result = res
    return outp



# revision 17
# speedup vs baseline: 1.0989x; 1.0989x over previous
"""Multi-head attention kernel for Trainium2, 8 NeuronCores, data-parallel over batch.

Problem: batch=16, pos=577, d_model=1024, n_heads=16, d_head=64, fp32.
Sharding: batch across 8 cores (2 batch items per core), no collectives.

v3: restructured phase B for ACT/LDW efficiency and parallel normalization.
  - q chunks (512, 65): one S stationary per (kt, hd) serves both chunks;
    exp instructions are [128,512] (q0) plus two batched strided exps
    covering all five kt's 65-wide q1 slots -> far fewer ACT instructions.
  - AV stationary is [V_h | ones64] (M=128) via a 2-chunk access pattern
    into a shared ones block: PSUM rows 0:64 = Z', rows 64:128 = softmax
    denominator replicated 64x. Normalization = 64-lane reciprocal +
    tensor-tensor multiply straight out of PSUM (no 1-partition ops, no
    DRAM-roundtrip broadcasts).
  - b_V folded into the V projection eviction (Z'/D = PV/D + b_V exactly);
    b_Q/b_K fused in Q/K evicts; b_O fused in the C-phase evict.
  - Tail: emit_attn(0,7) -> C(b=0) -> emit_attn(1,7) -> C(b=1) so the PE
    stays warm through the B->C transition.

Per-core pipeline (B=2 local batch, T=577, D=1024, H=16, E=64):
  A) V first: V+[ones] groups per (b, T-tile) in sbVg; Q^T,K^T as
     [HE, B*T] bf16 via interleaved m-tile projections.
  B) Per (batch, head-pair): S^T = K_h @ Q_h^T row-paired matmuls,
     P' = exp(S^T/8), Z'aug^T = [V_h|ones]^T @ P'^T -> [128, q];
     rows 64:128 = denominator; recip + multiply evict to sbZ bf16.
  C) Out = Z^T.T @ W_O (K=128 accumulation over 8 head-pairs) + b_O.
"""
import numpy as np

import concourse.bass as bass
import concourse.tile as tile
from concourse import bacc, mybir
from concourse.bass import AP

F32 = mybir.dt.float32
BF16 = mybir.dt.bfloat16
AF = mybir.ActivationFunctionType

NCORES = 8
_DEBUG_DUMPS = None
B = 2            # batch per core
T = 577
D = 1024
H = 16
E = 64
HE = H * E       # 1024
BT = B * T       # 1154

KT = 8                                   # k-tiles over D
MT = 8                                   # m-tiles over HE (head pairs)
A_N = [(0, 386), (386, 384), (770, 384)]  # bt chunks for phase A
TT = [(0, 128), (128, 128), (256, 128), (384, 128), (512, 65)]  # tiles over T
N512 = [(0, 512), (512, 512)]            # 512-chunks over HE / D
VGW = HE                                 # per (b,tile) V-group width
Q0 = 512                                 # q0 chunk width
Q1 = T - Q0                              # 65: q1 chunk width


def build_graph():
    nc = bacc.Bacc("TRN2", target_bir_lowering=False, debug=False,
                   num_devices=NCORES)

    xq = nc.dram_tensor("query_input", [D, BT], BF16, kind="ExternalInput")
    xk = nc.dram_tensor("key_input", [D, BT], BF16, kind="ExternalInput")
    xv = nc.dram_tensor("value_input", [D, BT], BF16, kind="ExternalInput")
    wq = nc.dram_tensor("W_Q", [D, HE], BF16, kind="ExternalInput")
    wk = nc.dram_tensor("W_K", [D, HE], BF16, kind="ExternalInput")
    wv = nc.dram_tensor("W_V", [D, HE], BF16, kind="ExternalInput")
    wo = nc.dram_tensor("W_O", [HE, D], BF16, kind="ExternalInput")
    bq = nc.dram_tensor("b_Q", [128, MT], F32, kind="ExternalInput")
    bk = nc.dram_tensor("b_K", [128, MT], F32, kind="ExternalInput")
    bv = nc.dram_tensor("b_V", [1, HE], BF16, kind="ExternalInput")
    bo = nc.dram_tensor("b_O", [1, D], BF16, kind="ExternalInput")
    out = nc.dram_tensor("out", [B, T, D], F32, kind="ExternalOutput")

    with tile.TileContext(nc) as tc:
        _body(nc, tc, xq, xk, xv, wq, wk, wv, wo, bq, bk, bv, bo, out)
    nc.compile()
    return nc


def _fp(t):
    """Partition row-pitch (elements) of a tile AP."""
    return t.ap[0][0]


def _body(nc, tc, xq, xk, xv, wq, wk, wv, wo, bq, bk, bv, bo, out):
    from contextlib import ExitStack
    _last_pp = [None]
    est = ExitStack()
    with est:
        # ---- persistent pools; packed tiles ----
        sbQ_p = est.enter_context(tc.tile_pool(name="sbQ", bufs=1))
        sbK_p = est.enter_context(tc.tile_pool(name="sbK", bufs=1))
        sbVg_p = est.enter_context(tc.tile_pool(name="sbVg", bufs=1))
        sbZ_p = est.enter_context(tc.tile_pool(name="sbZ", bufs=1))
        xt_p = est.enter_context(tc.tile_pool(name="xt", bufs=3))
        wt_p = est.enter_context(tc.tile_pool(name="wt", bufs=3))
        const_p = est.enter_context(tc.tile_pool(name="const", bufs=1))
        pp_p = est.enter_context(tc.tile_pool(name="pp", bufs=1))
        rpf_p = est.enter_context(tc.tile_pool(name="rpf", bufs=1))

        bqc = const_p.tile([128, MT], F32, tag="bqc")
        bkc = const_p.tile([128, MT], F32, tag="bkc")
        bvb = const_p.tile([128, HE], BF16, tag="bvb")
        boc = const_p.tile([128, D], BF16, tag="boc")
        ones = const_p.tile([128, E], BF16, tag="ones")
        nc.sync.dma_start(bqc[:], bq.ap())
        nc.sync.dma_start(bkc[:], bk.ap())
        nc.sync.dma_start(bvb[:], bv.ap().partition_broadcast(128))
        nc.sync.dma_start(boc[:], bo.ap().partition_broadcast(128))
        nc.vector.memset(ones[:], 1.0)

        # packed persistent tiles (bf16)
        sbQ = sbQ_p.tile([128, MT * BT], BF16, tag="sbQ")     # [:, m*BT + bt]
        sbK = sbK_p.tile([128, MT * BT], BF16, tag="sbK")
        sbVg = sbVg_p.tile([128, 10 * VGW], BF16, tag="sbVg")  # V heads packed
        sbZ = sbZ_p.tile([128, B * MT * T], BF16, tag="sbZ")  # [:, (b*MT+hp)*T + t]

        def zsl(b, hp, lo, sz, to, tsz):
            base = (b * MT + hp) * T
            return sbZ[lo:lo + sz, base + to:base + to + tsz]

        # ================= Phase A: projections =================
        def load_xw(x_in, w_in):
            xt = xt_p.tile([128, KT * BT], BF16, tag="xt", name="xt")
            wt = wt_p.tile([128, KT * HE], BF16, tag="wt", name="wt")
            for k in range(KT):
                nc.sync.dma_start(xt[:, k * BT:(k + 1) * BT],
                                  x_in.ap()[k * 128:(k + 1) * 128, :])
                nc.sync.dma_start(wt[:, k * HE:(k + 1) * HE],
                                  w_in.ap()[k * 128:(k + 1) * 128, :])
            return xt, wt

        # --- V first: frees its PSUM scope before the A/B overlap region ---
        with tc.tile_pool(name="psV", bufs=6, space="PSUM") as psV_p:
            xt, wt = load_xw(xv, wv)
            for b in range(B):
                for ti, (to, tsz) in enumerate(TT):
                    vbase = (b * 5 + ti) * VGW
                    bto = b * T + to
                    for ni, (no, nsz) in enumerate(N512):
                        ps = psV_p.tile([128, 512], F32, tag="psV", name="psV")
                        for k in range(KT):
                            nc.tensor.matmul(
                                ps[:tsz, :],
                                xt[:, k * BT + bto:k * BT + bto + tsz],
                                wt[:, k * HE + no:k * HE + no + nsz],
                                start=(k == 0), stop=(k == KT - 1))
                        # batched evict with b_V fold: heads contiguous
                        nc.vector.tensor_add(
                            sbVg[:tsz, vbase + no:vbase + no + nsz],
                            ps[:tsz, :], bvb[:tsz, no:no + nsz])

        # --- Q/K inputs + W_O (early, slots free after V) ---
        xtq, wtq = load_xw(xq, wq)
        xtk, wtk = load_xw(xk, wk)
        wot = wt_p.tile([128, MT * D], BF16, tag="wt", name="wot")
        for hp in range(MT):
            nc.sync.dma_start(wot[:, hp * D:(hp + 1) * D],
                              wo.ap()[hp * 128:(hp + 1) * 128, :])

        # ========== A/B overlap region pools ==========
        ab = ExitStack()
        psS_p = ab.enter_context(tc.tile_pool(name="psS", bufs=2, space="PSUM"))
        psq1_p = ab.enter_context(tc.tile_pool(name="psq1", bufs=1, space="PSUM"))
        psq1b_p = ab.enter_context(tc.tile_pool(name="psq1b", bufs=1, space="PSUM"))
        psZ_p = ab.enter_context(tc.tile_pool(name="psZ", bufs=1, space="PSUM"))

        def emit_attn(b, hp):
            qb = hp * BT + b * T
            pzs = [psZ_p.tile([128, 512], F32, tag=f"psz{hd}", name=f"psz{hd}")
                   for hd in range(2)]
            pq1 = psq1_p.tile([128, 512], F32, tag="psq1", name="pq1")
            pq1b = psq1b_p.tile([128, 325], F32, tag="psq1b", name="pq1b")
            pp = pp_p.tile([128, 5 * BT], BF16, tag="pp", name="pp")
            _last_pp[0] = pp
            # ---- S + exp(q0) per (kt, hd); S q1 into packed slots ----
            for kt, (ko, ksz) in enumerate(TT):
                for hd in range(2):
                    lo = hd * 64
                    ps = psS_p.tile([128, 512], F32, tag="psS", name="psS")
                    statK = sbK[lo:lo + 64, qb + ko:qb + ko + ksz]
                    nc.tensor.matmul(
                        ps[:ksz, :],
                        statK,
                        sbQ[lo:lo + 64, qb:qb + Q0],
                        start=True, stop=True, tile_position=(lo, 0))
                    q1dst = (pq1[:ksz, 130 + kt * 65:130 + kt * 65 + 65]
                             if hd == 0 else
                             pq1b[:ksz, kt * 65:kt * 65 + 65])
                    nc.tensor.matmul(
                        q1dst,
                        statK,
                        sbQ[lo:lo + 64, qb + Q0:qb + T],
                        start=True, stop=True, tile_position=(lo, 0))
                    nc.scalar.activation(
                        pp[:ksz, kt * BT + hd * T:kt * BT + hd * T + Q0],
                        ps[:ksz, :], AF.Exp, scale=0.125)
            # ---- batched q1 exps: one strided instr per hd over 5 kt ----
            ppv = pp[:, :].rearrange("p (k c) -> p k c", c=BT)
            for hd in range(2):
                src = (pq1[:, 130:455] if hd == 0 else pq1b[:, 0:325])
                src = src.rearrange("p (k c) -> p k c", c=65)
                dst = ppv[:, :, hd * T + Q0:hd * T + Q0 + 65]
                nc.scalar.activation(dst, src, AF.Exp, scale=0.125)
            # ---- AV q0: accumulate over kt; denom rows 0:64 (base-0 for the
            #      recip custom op), Z' rows 64:128. Concurrent col-tiles;
            #      has_written bits: only the bank's first MM uses start=True;
            #      later chain-openers rely on cleared bits -> overwrite+set.
            for kt, (ko, ksz) in enumerate(TT):
                for hd in range(2):
                    h = 2 * hp + hd
                    vcol = (b * 5 + kt) * VGW + h * E
                    mv = pp[:ksz, kt * BT + hd * T:kt * BT + hd * T + Q0]
                    nc.tensor.matmul(
                        pzs[hd][0:64, :], ones[:ksz, :], mv,
                        start=(kt == 0), stop=(kt == 4),
                        tile_position=(0, 0), skip_group_check=True)
                    nc.tensor.matmul(
                        pzs[hd][64:128, :], sbVg[:ksz, vcol:vcol + E], mv,
                        start=(kt == 0), stop=(kt == 4),
                        tile_position=(0, 64), skip_group_check=True)
            # ---- AV q1 (after q1 exps; N=65 each). hd-outer: the two
            #      pair-chains sharing this bank must open sequentially.
            for hd in range(2):
                for kt, (ko, ksz) in enumerate(TT):
                    h = 2 * hp + hd
                    vcol = (b * 5 + kt) * VGW + h * E
                    mv = pp[:ksz,
                            kt * BT + hd * T + Q0:kt * BT + hd * T + T]
                    nc.tensor.matmul(
                        pq1[0:64, hd * 65:hd * 65 + 65],
                        ones[:ksz, :], mv,
                        start=(kt == 0), stop=(kt == 4),
                        tile_position=(0, 0), skip_group_check=True)
                    nc.tensor.matmul(
                        pq1[64:128, hd * 65:hd * 65 + 65],
                        sbVg[:ksz, vcol:vcol + E], mv,
                        start=(kt == 0), stop=(kt == 4),
                        tile_position=(0, 64), skip_group_check=True)
            # ---- normalize: recip of replicated denom rows, mul-evict ----
            for hd in range(2):
                lo = hd * 64
                rpf = rpf_p.tile([64, 584], F32, tag=f"rpf{hd}",
                                 name=f"rpf{hd}")
                nc.vector.reciprocal_approx_fast(
                    rpf[:, 0:Q0], pzs[hd][0:64, :])
                nc.vector.reciprocal_approx_fast(
                    rpf[:, Q0:T], pq1[0:64, hd * 65:hd * 65 + 65])
                nc.vector.tensor_mul(
                    zsl(b, hp, lo, 64, 0, Q0), pzs[hd][64:128, :],
                    rpf[:, 0:Q0])
                nc.vector.tensor_mul(
                    zsl(b, hp, lo, 64, Q0, Q1),
                    pq1[64:128, hd * 65:hd * 65 + 65], rpf[:, Q0:T])

        # ---- interleaved Q/K projections + phase B ----
        with tc.tile_pool(name="psA", bufs=2, space="PSUM") as psA_p:
            for m in range(MT):
                for (xt, wt, b_col, dest) in ((xtq, wtq, bqc, sbQ),
                                              (xtk, wtk, bkc, sbK)):
                    for (no, nsz) in A_N:
                        ps = psA_p.tile([128, 386], F32, tag="psA", name="psA")
                        for k in range(KT):
                            nc.tensor.matmul(
                                ps[:, :nsz],
                                wt[:, k * HE + m * 128:k * HE + (m + 1) * 128],
                                xt[:, k * BT + no:k * BT + no + nsz],
                                start=(k == 0), stop=(k == KT - 1))
                        nc.vector.tensor_scalar_add(
                            dest[:, m * BT + no:m * BT + no + nsz],
                            ps[:, :nsz], b_col[:, m:m + 1])
                if m < MT - 1:
                    for b in range(B):
                        emit_attn(b, m)
        # psA closed: 2 banks free for psO

        # ================= Phase C: output projection =================
        sbO_p = est.enter_context(tc.tile_pool(name="sbO", bufs=3))

        def emit_out(b, psO_p):
            for (mo, msz) in TT:
                for (no, nsz) in N512:
                    ps = psO_p.tile([128, 512], F32, tag="psO", name="psO")
                    for hp in range(MT):
                        nc.tensor.matmul(
                            ps[:msz, :],
                            zsl(b, hp, 0, 128, mo, msz),
                            wot[:, hp * D + no:hp * D + no + nsz],
                            start=(hp == 0), stop=(hp == MT - 1))
                    so = sbO_p.tile([128, 512], F32, tag="sbO", name="sbO")
                    nc.vector.tensor_add(so[:msz, :], ps[:msz, :],
                                         boc[:msz, no:no + nsz])
                    nc.sync.dma_start(
                        out.ap()[b, mo:mo + msz, no:no + nsz], so[:msz, :])

        with tc.tile_pool(name="psO", bufs=2, space="PSUM") as psO_p:
            emit_attn(0, MT - 1)
            emit_out(0, psO_p)
            emit_attn(1, MT - 1)
            emit_out(1, psO_p)
        ab.close()

        if _DEBUG_DUMPS is not None:
            for nm, t in (("dbg_sbQ", sbQ), ("dbg_sbK", sbK),
                          ("dbg_sbVg", sbVg), ("dbg_sbZ", sbZ),
                          ("dbg_pp", _last_pp[0])):
                d = nc.dram_tensor(nm, list(t.shape), BF16,
                                   kind="ExternalOutput")
                nc.sync.dma_start(d.ap(), t[:, :])


_GRAPH = None


def _get_graph():
    global _GRAPH
    if _GRAPH is None:
        _GRAPH = build_graph()
    return _GRAPH


def kernel(query_input, key_input, value_input, W_Q, W_K, W_V, W_O,
           b_Q, b_K, b_V, b_O, _trace=False, _trace_kwargs=None):
    import ml_dtypes
    from concourse.bass_utils import run_bass_kernel_spmd

    nc = _get_graph()
    f = np.ascontiguousarray
    bf = ml_dtypes.bfloat16

    def xT(x, sl):
        x = np.asarray(x[sl], np.float32)
        return f(x.reshape(B * T, D).T.astype(bf))

    def wT(w):
        w = np.asarray(w, np.float32)
        return f(w.transpose(1, 0, 2).reshape(D, HE).astype(bf))

    def bcol(bx):
        bx = np.asarray(bx, np.float32).reshape(HE)
        return f(bx.reshape(MT, 128).T)

    wq_m, wk_m, wv_m = wT(W_Q), wT(W_K), wT(W_V)
    wo_m = f(np.asarray(W_O, np.float32).reshape(HE, D).astype(bf))
    bq_m, bk_m = bcol(b_Q), bcol(b_K)
    bv_m = f(np.asarray(b_V, np.float32).reshape(1, HE).astype(bf))
    bo_m = f(np.asarray(b_O, np.float32).reshape(1, D).astype(bf))
    in_maps = []
    for c in range(NCORES):
        sl = slice(2 * c, 2 * c + 2)
        in_maps.append({
            "query_input": xT(query_input, sl),
            "key_input": xT(key_input, sl),
            "value_input": xT(value_input, sl),
            "W_Q": wq_m,
            "W_K": wk_m,
            "W_V": wv_m,
            "W_O": wo_m,
            "b_Q": bq_m,
            "b_K": bk_m,
            "b_V": bv_m,
            "b_O": bo_m,
        })
    res = run_bass_kernel_spmd(nc, in_maps, core_ids=list(range(NCORES)),
                               trace=_trace, **(_trace_kwargs or {}))
    outp = np.concatenate([res.results[c]["out"] for c in range(NCORES)], axis=0)
    if _trace:
        kernel._last_result = res
    return outp


# revision 19
# speedup vs baseline: 1.1257x; 1.0244x over previous
"""Multi-head attention kernel for Trainium2, 8 NeuronCores, data-parallel over batch.

Problem: batch=16, pos=577, d_model=1024, n_heads=16, d_head=64, fp32.
Sharding: batch across 8 cores (2 batch items per core), no collectives.

v4: phase B restructured around big exp instructions and merged AV matmuls.
  - q chunks (512, 65): one S stationary per (kt, hd) serves both chunks;
    exp instructions are [128,512] (q0) plus one batched strided exp per hd
    covering all five kt's 65-wide q1 slots.
  - AV stationary is [ones64 | V_h] (M=128, contiguous, FWL-eligible):
    PSUM rows 0:64 = softmax denominator replicated 64x, rows 64:128 = Z'.
    Normalization = 64-lane reciprocal_approx_fast (base-0 only!) +
    tensor-tensor multiply straight out of PSUM.
  - b_V folded into the V projection eviction (Z'/D = PV/D + b_V exactly);
    b_Q/b_K fused in Q/K evicts; b_O fused in the C-phase evict.
  - Tail: emit_attn(0,7) -> C(b=0) -> emit_attn(1,7) -> C(b=1) keeps the PE
    warm through the B->C transition.

PSUM banks (8): psS 2 (rotating S q0 staging), pq1a 1 (S-q1 slots hd0),
pq1b 1 (S-q1 slots hd1 + both AV-q1 chains, sequential), psZ 2 (AV-q0
accumulators per hd), psA 2 (Q/K projection staging).

has_written semantics learned the hard way: accumulation chains sharing a
bank must not interleave their start=True openers; sequential chains and
write-once groups are safe.
"""
import numpy as np

import concourse.bass as bass
import concourse.tile as tile
from concourse import bacc, mybir

F32 = mybir.dt.float32
BF16 = mybir.dt.bfloat16
AF = mybir.ActivationFunctionType

NCORES = 8
_DEBUG_DUMPS = None
B = 2            # batch per core
T = 577
D = 1024
H = 16
E = 64
HE = H * E       # 1024
BT = B * T       # 1154

KT = 8                                   # k-tiles over D
MT = 8                                   # m-tiles over HE (head pairs)
A_N = [(0, 386), (386, 384), (770, 384)]  # bt chunks for phase A
TT = [(0, 128), (128, 128), (256, 128), (384, 128), (512, 65)]  # tiles over T
N512 = [(0, 512), (512, 512)]            # 512-chunks over HE / D
VGW = H * 128                            # 2048: per (b,tile) [ones|V] groups
Q0 = 512                                 # q0 chunk width
Q1 = T - Q0                              # 65: q1 chunk width


def build_graph():
    nc = bacc.Bacc("TRN2", target_bir_lowering=False, debug=False,
                   num_devices=NCORES)

    xq = nc.dram_tensor("query_input", [D, BT], BF16, kind="ExternalInput")
    xk = nc.dram_tensor("key_input", [D, BT], BF16, kind="ExternalInput")
    xv = nc.dram_tensor("value_input", [D, BT], BF16, kind="ExternalInput")
    wq = nc.dram_tensor("W_Q", [D, HE], BF16, kind="ExternalInput")
    wk = nc.dram_tensor("W_K", [D, HE], BF16, kind="ExternalInput")
    wv = nc.dram_tensor("W_V", [D, HE], BF16, kind="ExternalInput")
    wo = nc.dram_tensor("W_O", [HE, D], BF16, kind="ExternalInput")
    bq = nc.dram_tensor("b_Q", [128, MT], F32, kind="ExternalInput")
    bk = nc.dram_tensor("b_K", [128, MT], F32, kind="ExternalInput")
    bv = nc.dram_tensor("b_V", [1, HE], BF16, kind="ExternalInput")
    bo = nc.dram_tensor("b_O", [1, D], BF16, kind="ExternalInput")
    out = nc.dram_tensor("out", [B, T, D], F32, kind="ExternalOutput")

    with tile.TileContext(nc) as tc:
        _body(nc, tc, xq, xk, xv, wq, wk, wv, wo, bq, bk, bv, bo, out)
    nc.compile()
    return nc


def _body(nc, tc, xq, xk, xv, wq, wk, wv, wo, bq, bk, bv, bo, out):
    from contextlib import ExitStack
    _last_pp = [None]
    est = ExitStack()
    with est:
        # ---- persistent pools; packed tiles ----
        sbQ_p = est.enter_context(tc.tile_pool(name="sbQ", bufs=1))
        sbK_p = est.enter_context(tc.tile_pool(name="sbK", bufs=1))
        sbVg_p = est.enter_context(tc.tile_pool(name="sbVg", bufs=1))
        sbZ_p = est.enter_context(tc.tile_pool(name="sbZ", bufs=1))
        xt_p = est.enter_context(tc.tile_pool(name="xt", bufs=2))
        wt_p = est.enter_context(tc.tile_pool(name="wt", bufs=3))
        const_p = est.enter_context(tc.tile_pool(name="const", bufs=1))

        bqc = const_p.tile([128, MT], F32, tag="bqc")
        bkc = const_p.tile([128, MT], F32, tag="bkc")
        bvb = const_p.tile([128, HE], BF16, tag="bvb")
        boc = const_p.tile([128, D], BF16, tag="boc")
        nc.sync.dma_start(bqc[:], bq.ap())
        nc.sync.dma_start(bkc[:], bk.ap())
        nc.sync.dma_start(bvb[:], bv.ap().partition_broadcast(128))
        nc.sync.dma_start(boc[:], bo.ap().partition_broadcast(128))

        # packed persistent tiles (bf16)
        sbQ = sbQ_p.tile([128, MT * BT], BF16, tag="sbQ")     # [:, m*BT + bt]
        sbK = sbK_p.tile([128, MT * BT], BF16, tag="sbK")
        # per (b,tile): 16 head-groups of [ones64 | V_h 64]
        sbVg = sbVg_p.tile([128, 10 * VGW], BF16, tag="sbVg")
        sbZ = sbZ_p.tile([128, B * MT * T], BF16, tag="sbZ")  # [:, (b*MT+hp)*T + t]

        def zsl(b, hp, lo, sz, to, tsz):
            base = (b * MT + hp) * T
            return sbZ[lo:lo + sz, base + to:base + to + tsz]

        # ================= Phase A: projections =================
        def load_xw(x_in, w_in, pool):
            xt = pool.tile([128, KT * BT], BF16, tag="xt", name="xt")
            wt = wt_p.tile([128, KT * HE], BF16, tag="wt", name="wt")
            for k in range(KT):
                nc.sync.dma_start(xt[:, k * BT:(k + 1) * BT],
                                  x_in.ap()[k * 128:(k + 1) * 128, :])
                nc.sync.dma_start(wt[:, k * HE:(k + 1) * HE],
                                  w_in.ap()[k * 128:(k + 1) * 128, :])
            return xt, wt

        # --- V first: scoped pools free both PSUM and xtv SBUF space ---
        with tc.tile_pool(name="xtv", bufs=1) as xtv_p, \
             tc.tile_pool(name="psV", bufs=6, space="PSUM") as psV_p:
            xt, wt = load_xw(xv, wv, xtv_p)
            for b in range(B):
                for ti, (to, tsz) in enumerate(TT):
                    vbase = (b * 5 + ti) * VGW
                    bto = b * T + to
                    # ones blocks for all 16 heads of this tile (gpsimd)
                    og = sbVg[:tsz, vbase:vbase + VGW].rearrange(
                        "p (h c) -> p h c", c=128)
                    nc.gpsimd.memset(og[:, :, 0:E], 1.0)
                    for ni, (no, nsz) in enumerate(N512):
                        ps = psV_p.tile([128, 512], F32, tag="psV", name="psV")
                        for k in range(KT):
                            nc.tensor.matmul(
                                ps[:tsz, :],
                                xt[:, k * BT + bto:k * BT + bto + tsz],
                                wt[:, k * HE + no:k * HE + no + nsz],
                                start=(k == 0), stop=(k == KT - 1))
                        # strided evict with b_V fold into [ones|V] groups
                        dst = sbVg[:tsz, vbase + ni * 8 * 128:
                                   vbase + (ni * 8 + 8) * 128].rearrange(
                            "p (h c) -> p h c", c=128)[:, :, E:128]
                        nc.vector.tensor_add(
                            dst,
                            ps[:tsz, :].rearrange("p (h c) -> p h c", c=E),
                            bvb[:tsz, no:no + nsz].rearrange(
                                "p (h c) -> p h c", c=E))

        # --- Q/K inputs + W_O (early; wot reuses wtv's slot) ---
        xtq, wtq = load_xw(xq, wq, xt_p)
        xtk, wtk = load_xw(xk, wk, xt_p)
        wot = wt_p.tile([128, MT * D], BF16, tag="wt", name="wot")
        for hp in range(MT):
            nc.sync.dma_start(wot[:, hp * D:(hp + 1) * D],
                              wo.ap()[hp * 128:(hp + 1) * 128, :])

        # ========== A/B overlap region pools ==========
        ab = ExitStack()
        pp_p = ab.enter_context(tc.tile_pool(name="pp", bufs=1))
        rpf_p = ab.enter_context(tc.tile_pool(name="rpf", bufs=1))
        psS_p = ab.enter_context(tc.tile_pool(name="psS", bufs=2, space="PSUM"))
        pq1a_p = ab.enter_context(tc.tile_pool(name="pq1a", bufs=1, space="PSUM"))
        pq1b_p = ab.enter_context(tc.tile_pool(name="pq1b", bufs=1, space="PSUM"))
        psZ_p = ab.enter_context(tc.tile_pool(name="psZ", bufs=1, space="PSUM"))

        def emit_attn(b, hp):
            qb = hp * BT + b * T
            pzs = [psZ_p.tile([128, 512], F32, tag=f"psz{hd}", name=f"psz{hd}")
                   for hd in range(2)]
            pq1a = pq1a_p.tile([128, 512], F32, tag="pq1a", name="pq1a")
            pq1b = pq1b_p.tile([128, 512], F32, tag="pq1b", name="pq1b")
            pp = pp_p.tile([128, 5 * BT], BF16, tag="pp", name="pp")
            _last_pp[0] = pp
            # ---- S + exp(q0) per (kt, hd); S q1 into packed slots ----
            for kt, (ko, ksz) in enumerate(TT):
                for hd in range(2):
                    lo = hd * 64
                    ps = psS_p.tile([128, 512], F32, tag="psS", name="psS")
                    statK = sbK[lo:lo + 64, qb + ko:qb + ko + ksz]
                    nc.tensor.matmul(
                        ps[:ksz, :],
                        statK,
                        sbQ[lo:lo + 64, qb:qb + Q0],
                        start=True, stop=True, tile_position=(lo, 0))
                    q1dst = (pq1a if hd == 0 else pq1b)[
                        :ksz, kt * 65:kt * 65 + 65]
                    nc.tensor.matmul(
                        q1dst,
                        statK,
                        sbQ[lo:lo + 64, qb + Q0:qb + T],
                        start=True, stop=True, tile_position=(lo, 0))
                    nc.scalar.activation(
                        pp[:ksz, kt * BT + hd * T:kt * BT + hd * T + Q0],
                        ps[:ksz, :], AF.Exp, scale=0.125)
            # ---- batched q1 exps: one strided instr per hd over 5 kt ----
            ppv = pp[:, :].rearrange("p (k c) -> p k c", c=BT)
            for hd in range(2):
                src = (pq1a if hd == 0 else pq1b)[:, 0:325]
                src = src.rearrange("p (k c) -> p k c", c=65)
                dst = ppv[:, :, hd * T + Q0:hd * T + Q0 + 65]
                nc.scalar.activation(dst, src, AF.Exp, scale=0.125)
            # ---- AV q0: merged [ones|V] stationary; D rows 0:64, Z' 64:128
            for kt, (ko, ksz) in enumerate(TT):
                for hd in range(2):
                    h = 2 * hp + hd
                    vcol = (b * 5 + kt) * VGW + h * 128
                    nc.tensor.matmul(
                        pzs[hd][:, :],
                        sbVg[:ksz, vcol:vcol + 128],
                        pp[:ksz, kt * BT + hd * T:kt * BT + hd * T + Q0],
                        start=(kt == 0), stop=(kt == 4))
            # ---- AV q1: single chains in pq1b spare cols, hd sequential ----
            for hd in range(2):
                for kt, (ko, ksz) in enumerate(TT):
                    h = 2 * hp + hd
                    vcol = (b * 5 + kt) * VGW + h * 128
                    nc.tensor.matmul(
                        pq1b[:, 325 + hd * 65:325 + hd * 65 + 65],
                        sbVg[:ksz, vcol:vcol + 128],
                        pp[:ksz,
                           kt * BT + hd * T + Q0:kt * BT + hd * T + T],
                        start=(kt == 0), stop=(kt == 4),
                        skip_group_check=True)
            # ---- normalize: recip of replicated denom rows, mul-evict ----
            for hd in range(2):
                lo = hd * 64
                rpf = rpf_p.tile([64, 584], F32, tag=f"rpf{hd}",
                                 name=f"rpf{hd}")
                nc.vector.reciprocal_approx_fast(
                    rpf[:, 0:Q0], pzs[hd][0:64, :])
                nc.vector.reciprocal_approx_fast(
                    rpf[:, Q0:T], pq1b[0:64, 325 + hd * 65:325 + hd * 65 + 65])
                nc.vector.tensor_mul(
                    zsl(b, hp, lo, 64, 0, Q0), pzs[hd][64:128, :],
                    rpf[:, 0:Q0])
                nc.vector.tensor_mul(
                    zsl(b, hp, lo, 64, Q0, Q1),
                    pq1b[64:128, 325 + hd * 65:325 + hd * 65 + 65],
                    rpf[:, Q0:T])

        # ---- interleaved Q/K projections + phase B ----
        with tc.tile_pool(name="psA", bufs=2, space="PSUM") as psA_p:
            for m in range(MT):
                for (xt, wt, b_col, dest) in ((xtq, wtq, bqc, sbQ),
                                              (xtk, wtk, bkc, sbK)):
                    for (no, nsz) in A_N:
                        ps = psA_p.tile([128, 386], F32, tag="psA", name="psA")
                        for k in range(KT):
                            nc.tensor.matmul(
                                ps[:, :nsz],
                                wt[:, k * HE + m * 128:k * HE + (m + 1) * 128],
                                xt[:, k * BT + no:k * BT + no + nsz],
                                start=(k == 0), stop=(k == KT - 1))
                        nc.vector.tensor_scalar_add(
                            dest[:, m * BT + no:m * BT + no + nsz],
                            ps[:, :nsz], b_col[:, m:m + 1])
                if m < MT - 1:
                    for b in range(B):
                        emit_attn(b, m)
        # psA closed: 2 banks free for psO

        # ================= Phase C: output projection =================
        sbO_p = ab.enter_context(tc.tile_pool(name="sbO", bufs=3))

        def emit_out(b, psO_p):
            for (mo, msz) in TT:
                for (no, nsz) in N512:
                    ps = psO_p.tile([128, 512], F32, tag="psO", name="psO")
                    for hp in range(MT):
                        nc.tensor.matmul(
                            ps[:msz, :],
                            zsl(b, hp, 0, 128, mo, msz),
                            wot[:, hp * D + no:hp * D + no + nsz],
                            start=(hp == 0), stop=(hp == MT - 1))
                    so = sbO_p.tile([128, 512], F32, tag="sbO", name="sbO")
                    nc.vector.tensor_add(so[:msz, :], ps[:msz, :],
                                         boc[:msz, no:no + nsz])
                    nc.sync.dma_start(
                        out.ap()[b, mo:mo + msz, no:no + nsz], so[:msz, :])

        with tc.tile_pool(name="psO", bufs=2, space="PSUM") as psO_p:
            emit_attn(0, MT - 1)
            emit_out(0, psO_p)
            emit_attn(1, MT - 1)
            emit_out(1, psO_p)

        if _DEBUG_DUMPS is not None:
            for nm, t in (("dbg_sbQ", sbQ), ("dbg_sbK", sbK),
                          ("dbg_sbVg", sbVg), ("dbg_sbZ", sbZ),
                          ("dbg_pp", _last_pp[0])):
                d = nc.dram_tensor(nm, list(t.shape), BF16,
                                   kind="ExternalOutput")
                nc.sync.dma_start(d.ap(), t[:, :])
        ab.close()


_GRAPH = None


def _get_graph():
    global _GRAPH
    if _GRAPH is None:
        _GRAPH = build_graph()
    return _GRAPH


def kernel(query_input, key_input, value_input, W_Q, W_K, W_V, W_O,
           b_Q, b_K, b_V, b_O, _trace=False, _trace_kwargs=None):
    import ml_dtypes
    from concourse.bass_utils import run_bass_kernel_spmd

    nc = _get_graph()
    f = np.ascontiguousarray
    bf = ml_dtypes.bfloat16

    def xT(x, sl):
        x = np.asarray(x[sl], np.float32)
        return f(x.reshape(B * T, D).T.astype(bf))

    def wT(w):
        w = np.asarray(w, np.float32)
        return f(w.transpose(1, 0, 2).reshape(D, HE).astype(bf))

    def bcol(bx):
        bx = np.asarray(bx, np.float32).reshape(HE)
        return f(bx.reshape(MT, 128).T)

    wq_m, wk_m, wv_m = wT(W_Q), wT(W_K), wT(W_V)
    wo_m = f(np.asarray(W_O, np.float32).reshape(HE, D).astype(bf))
    bq_m, bk_m = bcol(b_Q), bcol(b_K)
    bv_m = f(np.asarray(b_V, np.float32).reshape(1, HE).astype(bf))
    bo_m = f(np.asarray(b_O, np.float32).reshape(1, D).astype(bf))
    in_maps = []
    for c in range(NCORES):
        sl = slice(2 * c, 2 * c + 2)
        in_maps.append({
            "query_input": xT(query_input, sl),
            "key_input": xT(key_input, sl),
            "value_input": xT(value_input, sl),
            "W_Q": wq_m,
            "W_K": wk_m,
            "W_V": wv_m,
            "W_O": wo_m,
            "b_Q": bq_m,
            "b_K": bk_m,
            "b_V": bv_m,
            "b_O": bo_m,
        })
    res = run_bass_kernel_spmd(nc, in_maps, core_ids=list(range(NCORES)),
                               trace=_trace, **(_trace_kwargs or {}))
    outp = np.concatenate([res.results[c]["out"] for c in range(NCORES)], axis=0)
    if _trace:
        kernel._last_result = res
    return outp
